# revision 82
# baseline (speedup 1.0000x reference)
"""GQA attention kernel for Trainium2, 8 NeuronCores.

Problem: B=2, T=2048, D=1024, 16 Q heads / 4 KV heads, head_dim=64, RoPE,
causal softmax, out-projection.

Sharding: 8 cores = 2 (batch) x 4 (KV group). Core c handles batch c//4 and
KV group g=c%4 (query heads 4g..4g+3). wq/wk/wv column-sharded, wo
row-sharded; the 4 partial outputs per batch are summed on the host.

On-chip layout: everything is kept transposed (head_dim on partitions):
  xT (D, T), qT (256, T), kT (64, T).  Scores are computed directly in
transposed orientation scoresT[j, i] = k_j . q_i (j on partitions), so no
on-chip transposes of the attention matrix are needed.  Softmax runs without
max-subtraction (scores are O(6) bounded), and the denominator L[i] is
obtained for free by augmenting V with a ones-column in the PV matmul.
RoPE pairs are de-interleaved via a host-side column permutation of wq/wk so
rotate-half applies; the interleave never needs to be undone because q and k
share the same permutation and V/out stay in natural order.

The whole matmul datapath runs in fp16 (11-bit mantissa, full PE rate);
PSUM accumulation stays fp32.  Softmax normalization is deferred: each
head's unnormalized PV output is copied to SBUF, the 1/L reciprocals
(reciprocal_approx_fast, DVE) and the broadcast-multiply run interleaved
under the NEXT head's score matmuls so the PE never stalls on them.
"""

import numpy as np
import sys

sys.path.insert(0, "/opt/trn_rl_repo")

from concourse import bass, bacc, mybir, tile  # noqa: E402
from concourse.bass_utils import run_bass_kernel_spmd  # noqa: E402

F32 = mybir.dt.float32
F32R = mybir.dt.float32r
F16 = mybir.dt.float16

B, T, D = 2, 2048, 1024
HD = 64                      # head dim
NQH = 4                      # query heads per core
QCOLS = NQH * HD             # 256
KC = D // 128                # 8 contraction chunks
NT = T // 128                # 16 row tiles
NC4 = T // 512               # 4 512-wide column chunks
N_CORES = 8

_cache = {}


def _r(ap):
    return ap.bitcast(F32R)


def build_nc():
    """Build the (SPMD-identical) single-core bass program."""
    nc = bacc.Bacc("TRN2", target_bir_lowering=False, debug=False)

    # All inputs are pre-packed on the host to the exact SBUF layout so
    # each tensor loads with a single contiguous DMA descriptor (the Sync
    # engine dispatches DMAs serially at ~0.6us each; fewer dispatches =
    # shorter critical path to the first matmul).
    xT_d = nc.declare_dram_parameter("xT", [128, NC4, KC, 512], F16,
                                     isOutput=False)
    wq_d = nc.declare_dram_parameter("wq", [128, KC, QCOLS], F16,
                                     isOutput=False)
    wk_d = nc.declare_dram_parameter("wk", [128, KC, HD], F16, isOutput=False)
    wv_d = nc.declare_dram_parameter("wv", [128, KC, HD], F16, isOutput=False)
    wo_d = nc.declare_dram_parameter("wo", [128, 2, D], F16, isOutput=False)
    cos_d = nc.declare_dram_parameter("cosf", [128, T], F16, isOutput=False)
    sin_d = nc.declare_dram_parameter("sinf", [128, T], F16, isOutput=False)
    msk_d = nc.declare_dram_parameter("msk", [128, 128], F32, isOutput=False)
    idn_d = nc.declare_dram_parameter("idn", [HD, HD], F16, isOutput=False)
    out_d = nc.declare_dram_parameter("out", [T, D], F16, isOutput=True)

    with tile.TileContext(nc) as tc:
        with (
            tc.tile_pool(name="sb", bufs=1) as sb,
            # opened before the xT scope so their SBUF never aliases the
            # xT chunks (otherwise the first exp/copy writes of the
            # attention phase wait for the last projection reads of xT).
            tc.tile_pool(name="aox", bufs=2) as aox,
            tc.tile_pool(name="at", bufs=8) as at_pool,
            tc.tile_pool(name="outp", bufs=3) as outp,
        ):
            wq = sb.tile([128, KC, QCOLS], F16, tag="wq")
            wk = sb.tile([128, KC, HD], F16, tag="wk")
            wv = sb.tile([128, KC, HD], F16, tag="wv")
            wo = sb.tile([128, 2, D], F16, tag="wo")
            cosf = sb.tile([128, T], F16, tag="cosf")
            sinf = sb.tile([128, T], F16, tag="sinf")
            # triangular [128,128] additive mask for the leading block of
            # each diagonal tile (same for every diagonal position).
            msk = sb.tile([128, 128], F32, tag="msk")
            # ones row placed at partition 64 to align with the L row of the
            # PV accumulator (engines need matching partition bases).
            ones = sb.tile([65, HD], F16, tag="ones")
            qT = [sb.tile([128, T], F16, tag=f"qT{hp}", name=f"qT{hp}")
                  for hp in range(2)]
            # kT duplicated into both partition halves so scores matmuls can
            # read it at base partition 0 (even heads) or 64 (odd heads).
            kT = sb.tile([128, T], F16, tag="kT")
            vT = sb.tile([64, T], F16, tag="vT")
            idn = sb.tile([HD, HD], F16, tag="idn")
            v = sb.tile([128, NT, HD + 1], F16, tag="v")
            ao = [sb.tile([128, T], F16, tag=f"ao{hp}", name=f"ao{hp}")
                  for hp in range(2)]
            # per-head 1/L rows, kept at partition 64 (PSUM L row base).
            # fp32 from the exact reciprocal, fp16 for the lb broadcast
            # matmul (fp32r matmuls cannot write PSUM at partition base 64).
            linv = sb.tile([65, NQH, T], F32, tag="linv")
            linvh = sb.tile([65, NQH, T], F16, tag="linvh")

            # wk first (the K projection runs first); everything else is
            # issued interleaved behind the xT chunks so it doesn't delay
            # the first matmul's DMA wait.
            nc.sync.dma_start(wk[:, :, :], wk_d[:, :, :])

            # V (host-side) and the L ones-column are both scaled by 1/128 so
            # the unnormalized PV values fit in fp16; the pv/L ratio is
            # unchanged.
            nc.gpsimd.memset(ones[64:65, :], 1.0)
            nc.gpsimd.memset(v[:, :, HD:HD + 1], 1.0 / 128.0)

            # --- projections (xT lives only here) ---
            with (
                tc.tile_pool(name="sbx", bufs=1) as sbx,
                tc.tile_pool(name="rope", bufs=1) as rope_pool,
                tc.tile_pool(name="ppsum", bufs=2, space="PSUM") as ppsum,
            ):
                # One tile per 512-column chunk (Tile deps are per-tile, so
                # the ci=0 projections start after ~1/4 of the xT transfer).
                xTc = [sbx.tile([128, KC, 512], F16, tag=f"xT{ci}",
                                name=f"xT{ci}") for ci in range(NC4)]
                # ci=0 split per k-chunk so the K projection's accumulation
                # chain starts as soon as chunk k=0 lands; later chunks in
                # k-pairs to pipeline arrival with consumption while keeping
                # the serial dispatch count low.
                for k in range(KC):
                    nc.sync.dma_start(xTc[0][:, k, :], xT_d[:, 0, k, :])
                for k in range(0, KC, 4):
                    nc.sync.dma_start(xTc[1][:, k:k + 4, :],
                                      xT_d[:, 1, k:k + 4, :])
                nc.sync.dma_start(wq[:, 0:4, :], wq_d[:, 0:4, :])
                nc.sync.dma_start(wq[:, 4:8, :], wq_d[:, 4:8, :])
                nc.sync.dma_start(wv[:, :, :], wv_d[:, :, :])
                nc.sync.dma_start(cosf[:], cos_d[:])
                nc.sync.dma_start(sinf[:], sin_d[:])
                for ci in range(2, NC4):
                    for k in range(0, KC, 4):
                        nc.sync.dma_start(xTc[ci][:, k:k + 4, :],
                                          xT_d[:, ci, k:k + 4, :])
                nc.sync.dma_start(idn[:], idn_d[:])
                nc.sync.dma_start(msk[:], msk_d[:])
                nc.sync.dma_start(wo[:, :, :], wo_d[:, :, :])

                def rope_inplace(q_ap, nrows):
                    """q = q*cos + rot_half(q)*sin, on de-interleaved rows."""
                    rot = rope_pool.tile([128, T], F16, tag="rot")
                    for blk in range(nrows // 64):
                        r0 = blk * 64
                        nc.sync.dma_start(rot[r0:r0 + 32, :],
                                          q_ap[r0 + 32:r0 + 64, :])
                        nc.sync.dma_start(rot[r0 + 32:r0 + 64, :],
                                          q_ap[r0:r0 + 32, :])
                    nc.vector.tensor_mul(q_ap[0:nrows, :], q_ap[0:nrows, :],
                                         cosf[0:nrows, :])
                    nc.vector.tensor_mul(rot[0:nrows, :], rot[0:nrows, :],
                                         sinf[0:nrows, :])
                    nc.vector.tensor_add(q_ap[0:nrows, :], q_ap[0:nrows, :],
                                         rot[0:nrows, :])

                # K first: its rope + duplication run on DVE/DMA while the
                # Q projections stream on the PE, so attention can start
                # during the V projection.
                pk = ppsum.tile([64, T], F32, tag="proj")
                for ci in range(NC4):
                    cs = slice(ci * 512, (ci + 1) * 512)
                    for k in range(KC):
                        nc.tensor.matmul(
                            pk[:, cs], wk[:, k, :], xTc[ci][:, k, :],
                            start=(k == 0), stop=(k == KC - 1))
                nc.scalar.copy(kT[0:64, :], pk[:])
                rope_inplace(kT[:], 64)
                nc.sync.dma_start(kT[64:128, :], kT[0:64, :])

                def q_proj(hp):
                    pq = ppsum.tile([128, T], F32, tag="proj",
                                    name=f"pq{hp}")
                    for ci in range(NC4):
                        cs = slice(ci * 512, (ci + 1) * 512)
                        for k in range(KC):
                            nc.tensor.matmul(
                                pq[:, cs],
                                wq[:, k, hp * 128:(hp + 1) * 128],
                                xTc[ci][:, k, :],
                                start=(k == 0), stop=(k == KC - 1))
                    nc.scalar.copy(qT[hp][:], pq[:])
                    rope_inplace(qT[hp][:], 128)

                def v_transposes(ci):
                    # lagged one ci behind the vT matmuls so the PE never
                    # waits on the vT PSUM->SBUF copy.
                    for tt in range(4):
                        t = ci * 4 + tt
                        ptv = ppsum.tile([128, HD], F16, tag="proj",
                                         name=f"ptv{t}")
                        nc.tensor.transpose(
                            ptv[:], vT[:, t * 128:(t + 1) * 128], idn[:])
                        nc.scalar.copy(v[:, t, 0:HD], ptv[:])

                q_proj(0)
                # V as vT (dense N=512 matmuls like kT), then 16 PE
                # transposes into the [key-position, head-dim] layout the
                # PV contraction needs.
                for ci in range(NC4):
                    pvT = ppsum.tile([64, 512], F32, tag="proj",
                                     name=f"pvT{ci}")
                    for k in range(KC):
                        nc.tensor.matmul(
                            pvT[:], wv[:, k, :], xTc[ci][:, k, :],
                            start=(k == 0), stop=(k == KC - 1))
                    nc.scalar.copy(vT[:, ci * 512:(ci + 1) * 512], pvT[:])
                    if ci > 0:
                        v_transposes(ci - 1)
                q_proj(1)
                v_transposes(NC4 - 1)

            # --- attention, one KV head (4 query heads) ---
            with (
                tc.tile_pool(name="pvpsum", bufs=3, space="PSUM") as pvp,
                tc.tile_pool(name="scpsum", bufs=4, space="PSUM") as scp,
                tc.tile_pool(name="lbpsum", bufs=1, space="PSUM") as lbp,
            ):
                def norm_chunk(h, ci):
                    """ao[h][:, ci] *= 1/L, via PE broadcast of linv."""
                    hp, hr = divmod(h, 2)
                    rows = slice(hr * 64, hr * 64 + 64)
                    cs = slice(ci * 512, (ci + 1) * 512)
                    lb = lbp.tile([128, 512], F32, tag="lb",
                                  name=f"lb{h}_{ci}")
                    nc.tensor.matmul(lb[rows, :], ones[64:65, :],
                                     linvh[64:65, h, cs],
                                     start=True, stop=True)
                    nc.vector.tensor_mul(ao[hp][rows, cs],
                                         ao[hp][rows, cs], lb[rows, :])

                def emit_norm(h):
                    for ci in range(NC4):
                        norm_chunk(h, ci)

                # Flat software-pipelined stream: the PE executes in order,
                # so emit the score matmul of tile i+2 BEFORE the PV matmul
                # of tile i.  Otherwise pv(i) stalls the PE on the
                # mask->exp chain of tile i, the PE accumulates ~1us gaps,
                # and the HAM clock gate keeps the PE at 1.2 GHz for the
                # whole attention phase.
                tiles = [(h, ci, tj)
                         for h in range(NQH)
                         for ci in range(NC4)
                         for tj in range((ci + 1) * 4)]
                pend = {}    # tile idx -> (at, lv)
                pv_tiles = {}  # (h, ci) -> pv_acc
                drained = []  # (drain tile idx, h, ci) awaiting norm
                # small deferred DVE ops (reciprocal/cast chunks), dripped
                # between score tiles so they never clog the DVE queue and
                # stall the mask->exp chain.
                drip = []

                def emit_sc(idx):
                    h, ci, tj = tiles[idx]
                    hp, hr = divmod(h, 2)
                    qrow = slice(hr * 64, hr * 64 + 64)
                    # causal trim: diagonal tile r only covers query
                    # columns >= 128*r within this 512 chunk.
                    r = tj - ci * 4
                    c0 = max(0, r * 128)
                    lv = slice(c0, 512)
                    sc = scp.tile([128, 512], F32, tag="sc")
                    nc.tensor.matmul(
                        sc[:, lv],
                        kT[qrow, tj * 128:(tj + 1) * 128],
                        qT[hp][qrow, ci * 512 + c0:(ci + 1) * 512],
                        start=True, stop=True)
                    if r >= 0:  # leading 128 cols: triangular mask
                        nc.vector.tensor_add(
                            sc[:, c0:c0 + 128], sc[:, c0:c0 + 128], msk[:, :])
                    at = at_pool.tile([128, 512], F16, tag="at")
                    nc.scalar.activation(
                        at[:, lv], sc[:, lv],
                        mybir.ActivationFunctionType.Exp, scale=0.125)
                    pend[idx] = (at, lv)
                    # don't add DVE drip work while emitting a ci=0 block:
                    # those tiles are all diagonal, so their exps are
                    # mask-chained on the DVE and extra queue depth there
                    # stalls the first PV matmuls of the head.
                    if drip and ci != 0:
                        drip.pop(0)[1]()

                def emit_pv(idx):
                    h, ci, tj = tiles[idx]
                    hp, hr = divmod(h, 2)
                    n_tj = (ci + 1) * 4
                    at, lv = pend.pop(idx)
                    if tj == 0:
                        pv_tiles[(h, ci)] = pvp.tile(
                            [HD + 1, 512], F32, tag="pv", name=f"pv{h}_{ci}")
                    pv_acc = pv_tiles[(h, ci)]
                    nc.tensor.matmul(
                        pv_acc[:, lv], v[:, tj, :], at[:, lv],
                        start=(tj == 0), stop=(tj == n_tj - 1),
                        skip_group_check=True)
                    if tj != n_tj - 1:
                        return
                    # block drain: 1/L for this chunk (L sits in pv_acc row
                    # 64, the ones-column of v_aug); unnormalized PV -> SBUF.
                    # (reciprocal_approx_fast is broken on real HW.)
                    cs = slice(ci * 512, (ci + 1) * 512)

                    def rchunk(h, ci, pv_acc, q):
                        def go():
                            c = slice(ci * 512 + q * 128,
                                      ci * 512 + (q + 1) * 128)
                            cp = slice(q * 128, (q + 1) * 128)
                            with nc.allow_low_precision(reason="fp32r linv"):
                                nc.vector.reciprocal(
                                    _r(linv[64:65, h, c]),
                                    pv_acc[HD:HD + 1, cp])
                        return go

                    def cchunk(h, ci, q):
                        def go():
                            c = slice(ci * 512 + q * 128,
                                      ci * 512 + (q + 1) * 128)
                            nc.vector.tensor_copy(linvh[64:65, h, c],
                                                  linv[64:65, h, c])
                        return go
                    for q in range(4):
                        drip.append(((h, ci), rchunk(h, ci, pv_acc, q)))
                        drip.append(((h, ci), cchunk(h, ci, q)))
                    drained.append((idx, h, ci))
                    if hr == 0:
                        nc.scalar.copy(ao[hp][0:64, cs], pv_acc[0:HD, :])
                    else:
                        tmp = aox.tile([64, 512], F16, tag="aotmp")
                        nc.scalar.copy(tmp[:], pv_acc[0:HD, :])
                        nc.sync.dma_start(ao[hp][64:128, cs], tmp[:])
                    del pv_tiles[(h, ci)]

                LOOKAHEAD = 3
                for i in range(min(LOOKAHEAD, len(tiles))):
                    emit_sc(i)
                for i, (h, ci, tj) in enumerate(tiles):
                    # normalization of a drained chunk rides under later
                    # score matmuls; the >=8-tile lag gives the dripped
                    # reciprocal/cast chunks time to finish on the DVE, so
                    # the lb broadcast never stalls the PE.  Any of its
                    # drips not yet popped must be emitted first (the lb
                    # read must come after the linvh writes in program
                    # order).
                    if drained and drained[0][0] <= i - 8:
                        _, hh, cc = drained.pop(0)
                        while drip and drip[0][0] == (hh, cc):
                            drip.pop(0)[1]()
                        norm_chunk(hh, cc)
                    if i + LOOKAHEAD < len(tiles):
                        emit_sc(i + LOOKAHEAD)
                    emit_pv(i)

                # --- epilogue: the last drained chunks normalize under the
                # first out-projection blocks ---
                for ci in range(NC4):
                    if ci == 1:
                        while drip:
                            drip.pop(0)[1]()
                        while drained:
                            _, hh, cc = drained.pop(0)
                            norm_chunk(hh, cc)
                    for t in range(4 * ci, 4 * ci + 4):
                        ot = outp.tile([128, D], F16, tag="ot",
                                       name=f"ot{t}")
                        for nh in range(2):
                            ns = slice(nh * 512, (nh + 1) * 512)
                            po = scp.tile([128, 512], F32, tag="sc",
                                          name=f"po{t}_{nh}")
                            for cc in range(2):
                                nc.tensor.matmul(
                                    po[:],
                                    ao[cc][:, t * 128:(t + 1) * 128],
                                    wo[:, cc, ns],
                                    start=(cc == 0), stop=(cc == 1))
                            # alternate engines so the PSUM->SBUF copies
                            # don't serialize on the scalar engine
                            if nh == 0:
                                nc.scalar.copy(ot[:, ns], po[:])
                            else:
                                nc.vector.tensor_copy(ot[:, ns], po[:])
                        nc.sync.dma_start(out_d[t * 128:(t + 1) * 128, :],
                                          ot[:])

    nc.compile()
    return nc


def make_in_maps(x, freqs_cos, freqs_sin, wq, wk, wv, wo):
    """Host-side sharding + layout prep. Returns per-core input dicts."""
    x = np.asarray(x, np.float32)
    fc = np.asarray(freqs_cos, np.float32)
    fs = np.asarray(freqs_sin, np.float32)
    wq = np.asarray(wq, np.float32)
    wk = np.asarray(wk, np.float32)
    wv = np.asarray(wv, np.float32)
    wo = np.asarray(wo, np.float32)

    perm = np.concatenate([np.arange(0, HD, 2), np.arange(1, HD, 2)])
    cosT = np.ascontiguousarray(fc.T)            # (32, T)
    sinT = np.ascontiguousarray(fs.T)
    cosf = np.concatenate([cosT] * 4, axis=0).astype(np.float16)
    sinf = np.concatenate([-sinT, sinT, -sinT, sinT], axis=0).astype(np.float16)

    jj = np.arange(128)[:, None]
    ii = np.arange(128)[None, :]
    mskT = np.where(jj <= ii, 0.0, -1e30).astype(np.float32)  # (128, 128)

    def pack_w(a):
        """(D, C) -> (128, KC, C): row k*128+p goes to [p, k, :]."""
        return np.ascontiguousarray(
            a.reshape(KC, 128, a.shape[1]).transpose(1, 0, 2))

    in_maps = []
    for c in range(N_CORES):
        b, g = divmod(c, 4)
        wq_c = wq[:, g * QCOLS:(g + 1) * QCOLS]
        wq_c = wq_c.reshape(D, NQH, HD)[:, :, perm].reshape(D, QCOLS)
        wk_c = wk[:, g * HD:(g + 1) * HD][:, perm]
        wv_c = wv[:, g * HD:(g + 1) * HD] * (1.0 / 128.0)
        wo_c = np.ascontiguousarray(
            wo[g * QCOLS:(g + 1) * QCOLS, :].reshape(2, 128, D)
            .transpose(1, 0, 2))
        # xT packed as [p, ci, k, cc]: element (d=k*128+p, t=ci*512+cc)
        xT_c = np.ascontiguousarray(
            x[b].T.reshape(KC, 128, NC4, 512).transpose(1, 2, 0, 3))
        in_maps.append({
            "xT": xT_c.astype(np.float16),
            "wq": pack_w(wq_c).astype(np.float16),
            "wk": pack_w(wk_c).astype(np.float16),
            "wv": pack_w(wv_c).astype(np.float16),
            "wo": wo_c.astype(np.float16),
            "cosf": cosf, "sinf": sinf, "msk": mskT,
            "idn": np.eye(HD, dtype=np.float16),
        })
    return in_maps


def run_on_cores(in_maps, trace=False, **kwargs):
    if "nc" not in _cache:
        _cache["nc"] = build_nc()
    return run_bass_kernel_spmd(
        _cache["nc"], in_maps, core_ids=list(range(N_CORES)), trace=trace,
        **kwargs)


def kernel(x, freqs_cos, freqs_sin, wq, wk, wv, wo):
    in_maps = make_in_maps(x, freqs_cos, freqs_sin, wq, wk, wv, wo)
    res = run_on_cores(in_maps)
    outs = [np.asarray(res.results[c]["out"], np.float32)
            for c in range(N_CORES)]
    full = np.empty((B, T, D), np.float32)
    for b in range(B):
        full[b] = outs[4 * b] + outs[4 * b + 1] + outs[4 * b + 2] + outs[4 * b + 3]
    return full
